# revision 1
# baseline (speedup 1.0000x reference)
"""Causal self-attention (flipped mask: attend to k >= q) on 8 Trainium2 cores.

Sharding: 2-way data parallel over batch x 4-way head parallel (4 heads/core).
Each core computes x[b] -> qkv (its 4 heads) -> attention -> partial out-proj
(its 256 rows of Wo); the host sums the 4 partials per batch (tensor-parallel
unshard) to produce the full [B, T, C] output.

Layout strategy on-core (all fp32 in memory, float32r for matmuls):
  - xT [C, T] built via PE transpose-mode (all projections contract over C
    and DRAM x is row-major in C).
  - qT/kT [d, T] per head (transposed projections, stationary W chunks).
  - v natural [T, d] per head, augmented with a ones column so the attention
    V-matmul also produces the softmax denominator in a spare PSUM partition.
  - scores computed TRANSPOSED: sT[kt, qt] = kT^T @ qT so softmax's reduction
    lands on the PSUM partition axis and is folded into the V-matmul; exp has
    no max-subtraction (scores are O(1) by construction, exp is safe in fp32).
  - flipped-causal mask applied additively (-1e5) on diagonal-band blocks
    only, via a host-precomputed sliding mask tile.
  - out-proj consumes yT directly as the stationary operand.
"""

import numpy as np

B, T, C = 2, 2048, 1024
H = 16
D = 64
NH = 4           # heads per core
HC = NH * D      # 256 local head cols
SCALE = 0.125    # 1/sqrt(D)
NEG = -1.0e5
N_CORES = 8

NT = T // 128    # 16 t-tiles
NCC = C // 128   # 8 c-chunks
NQ = T // 512    # 4 q-chunks of 512
NJ = T // 128    # 16 kt-chunks of 128

_CACHE = {}


def _build_nc():
    import concourse.tile as tile
    from concourse import bacc, mybir

    f32 = mybir.dt.float32
    f32r = mybir.dt.float32r
    f16 = mybir.dt.float16
    Exp = mybir.ActivationFunctionType.Exp
    Ident = mybir.ActivationFunctionType.Identity

    nc = bacc.Bacc(None, target_bir_lowering=False, debug=False)

    xb = nc.dram_tensor("xb", [T, C], f16, kind="ExternalInput")
    wq = nc.dram_tensor("wq", [C, HC], f16, kind="ExternalInput")
    wk = nc.dram_tensor("wk", [C, HC], f16, kind="ExternalInput")
    wv = nc.dram_tensor("wv", [C, HC], f16, kind="ExternalInput")
    bqs = nc.dram_tensor("bqs", [HC], f32, kind="ExternalInput")
    bk = nc.dram_tensor("bk", [HC], f32, kind="ExternalInput")
    bvb = nc.dram_tensor("bvb", [128, HC], f32, kind="ExternalInput")
    wo = nc.dram_tensor("wo", [HC, C], f16, kind="ExternalInput")
    bob = nc.dram_tensor("bob", [128, C], f32, kind="ExternalInput")
    mskB = nc.dram_tensor("mskB", [128, 896], f32, kind="ExternalInput")
    vones = nc.dram_tensor("vones", [128, 2, 195], f16, kind="ExternalInput")
    out = nc.dram_tensor("out", [T, C], f32, kind="ExternalOutput")

    def r(ap):
        return ap.bitcast(f32r)

    with tile.TileContext(nc) as tc, (
        tc.tile_pool(name="consts", bufs=1)) as consts, (
        tc.tile_pool(name="wts", bufs=1)) as wts, (
        tc.tile_pool(name="persist", bufs=1)) as persist:

        # ---- constants ----
        msk_sb = consts.tile([128, 896], f32)
        nc.sync.dma_start(out=msk_sb, in_=mskB[:, :])
        bq_sb = consts.tile([128, 2], f32)
        nc.sync.dma_start(out=bq_sb, in_=bqs.rearrange("(a p) -> p a", p=128))
        bk_sb = consts.tile([128, 2], f32)
        nc.sync.dma_start(out=bk_sb, in_=bk.rearrange("(a p) -> p a", p=128))
        bvb_sb = consts.tile([128, NH, D], f32)
        nc.sync.dma_start(out=bvb_sb, in_=bvb.rearrange("p (h d) -> p h d", h=NH))
        bob_sb = consts.tile([128, C], f32)
        nc.sync.dma_start(out=bob_sb, in_=bob[:, :])

        # ---- weights ----
        wq_sb = wts.tile([128, NCC, HC], f16)
        nc.sync.dma_start(out=wq_sb, in_=wq.rearrange("(a p) n -> p a n", p=128))
        wk_sb = wts.tile([128, NCC, HC], f16)
        nc.sync.dma_start(out=wk_sb, in_=wk.rearrange("(a p) n -> p a n", p=128))
        wv_sb = wts.tile([128, NCC, HC], f16)
        nc.sync.dma_start(out=wv_sb, in_=wv.rearrange("(a p) n -> p a n", p=128))
        wo_sb = wts.tile([128, 2, C], f16)
        nc.sync.dma_start(out=wo_sb, in_=wo.rearrange("(a p) n -> p a n", p=128))

        # ---- persistent activations ----
        qT_sb = persist.tile([128, 2, T], f16)   # [2 head-pair chunks, T]
        kT_sb = persist.tile([128, 2, T], f16)
        # v, augmented: per t-tile, per pair g: [65 even | 130 odd]
        # even block: cols 0..63 = v(2g), col 64 = 1.0
        # odd block:  col 0 = 1.0 (offset 65), cols 64..127 = v(2g+1)
        v_sb = persist.tile([128, NT, 2, 195], f16)
        yT_sb = persist.tile([128, 2, T], f16)

        for t0 in range(NT):
            nc.sync.dma_start(out=v_sb[:, t0, :, :], in_=vones[:, :, :])

        with (
            tc.tile_pool(name="xTp", bufs=1) as xTp,
            tc.tile_pool(name="psB", bufs=4, space="PSUM") as psB,
        ):
            xT_sb = xTp.tile([128, NCC, T], f16)

            # ---- phase A: transpose x via the DMA xbar (2-byte dtype) ----
            for c0 in range(NCC):
                nc.sync.dma_start_transpose(
                    out=xT_sb[:, c0, :], in_=xb[:, c0 * 128:(c0 + 1) * 128]
                )

            # ---- phase B: projections ----
            # Interleave one qk chain (N=512, MM hides its own LDW) with one
            # v chain (N=256, LDW of the per-(t0,c0) xT stationary would
            # otherwise be exposed) so weight loads hide under matmuls.
            for i in range(NT):
                gm, is_k = divmod(i, 2)
                g, m = divmod(gm, NQ)
                w_sb = wk_sb if is_k else wq_sb
                t0 = i
                psqk = psB.tile([128, 512], f32, tag="pj")
                psv = psB.tile([128, 512], f32, tag="pj")
                for c0 in range(NCC):
                    nc.tensor.matmul(
                        psqk,
                        lhsT=(w_sb[:, c0, g * 128:(g + 1) * 128]),
                        rhs=(xT_sb[:, c0, m * 512:(m + 1) * 512]),
                        start=(c0 == 0), stop=(c0 == NCC - 1),
                    )
                    nc.tensor.matmul(
                        psv[:, 0:HC],
                        lhsT=(xT_sb[:, c0, t0 * 128:(t0 + 1) * 128]),
                        rhs=(wv_sb[:, c0, :]),
                        start=(c0 == 0), stop=(c0 == NCC - 1),
                    )
                if is_k:
                    nc.scalar.activation(
                        kT_sb[:, g, m * 512:(m + 1) * 512], psqk, Ident,
                        bias=bk_sb[:, g:g + 1], scale=1.0,
                    )
                else:
                    nc.scalar.activation(
                        qT_sb[:, g, m * 512:(m + 1) * 512], psqk, Ident,
                        bias=bq_sb[:, g:g + 1], scale=SCALE,
                    )
                psv4 = psv[:, 0:HC].rearrange("p (h d) -> p h d", h=NH)
                for gg in range(2):
                    nc.vector.tensor_add(
                        v_sb[:, t0, gg, 0:64], psv4[:, 2 * gg, :], bvb_sb[:, 2 * gg, :]
                    )
                    nc.vector.tensor_add(
                        v_sb[:, t0, gg, 129:193], psv4[:, 2 * gg + 1, :],
                        bvb_sb[:, 2 * gg + 1, :],
                    )

        # ---- phases C/D ----
        with (
            tc.tile_pool(name="epool", bufs=8) as epool,
            tc.tile_pool(name="rpool", bufs=2) as rpool,
            tc.tile_pool(name="opool", bufs=2) as opool,
            tc.tile_pool(name="psS", bufs=4, space="PSUM") as psS,
            tc.tile_pool(name="psY", bufs=2, space="PSUM") as psY,
        ):
            # phase C: attention with transposed scores
            for g in range(2):
                for n in range(NQ):
                    qs = n * 512
                    # one 2-bank accumulator per group: even head in cols
                    # 0:512, odd head in cols 512:1024
                    yt = psY.tile([128, 1024], f32, tag="y")
                    ye = yt[:, 0:512]
                    yo = yt[:, 512:1024]

                    def emit_av(jj, eprev_e, eprev_o):
                        nc.tensor.matmul(
                            ye[0:65, :],
                            lhsT=(v_sb[:, jj, g, 0:65]),
                            rhs=(eprev_e[:, :]),
                            start=(jj == 4 * n), stop=(jj == NJ - 1),
                        )
                        nc.tensor.matmul(
                            yo,
                            lhsT=(v_sb[:, jj, g, 65:193]),
                            rhs=(eprev_o[:, :]),
                            start=(jj == 4 * n), stop=(jj == NJ - 1),
                        )

                    lag = []
                    for j in range(4 * n, NJ):
                        o = 128 * j - 512 * n
                        band = j < 4 * n + 4
                        ks = j * 128
                        ps_e = psS.tile([128, 512], f32, tag="s")
                        ps_o = psS.tile([128, 512], f32, tag="s")
                        nc.tensor.matmul(
                            ps_e,
                            lhsT=(kT_sb[0:64, g, ks:ks + 128]),
                            rhs=(qT_sb[0:64, g, qs:qs + 512]),
                            start=True, stop=True,
                        )
                        nc.tensor.matmul(
                            ps_o,
                            lhsT=(kT_sb[64:128, g, ks:ks + 128]),
                            rhs=(qT_sb[64:128, g, qs:qs + 512]),
                            start=True, stop=True,
                        )
                        if len(lag) >= 2:
                            emit_av(*lag.pop(0))
                        if band:
                            nc.vector.tensor_add(
                                ps_e, ps_e, msk_sb[:, 384 - o:896 - o],
                            )
                            nc.vector.tensor_add(
                                ps_o, ps_o, msk_sb[:, 384 - o:896 - o],
                            )
                        e_e = epool.tile([128, 512], f16, tag="e")
                        e_o = epool.tile([128, 512], f16, tag="e")
                        nc.scalar.activation(e_e, ps_e, Exp)
                        nc.scalar.activation(e_o, ps_o, Exp)
                        lag.append((j, e_e, e_o))
                    for item in lag:
                        emit_av(*item)
                    # normalize: denominators sit on 1 PSUM partition each
                    # (even @64, odd @0).  Make the reciprocal cheap by
                    # DMA-reshaping [1,512] -> [128,4] (recip cost scales with
                    # free size), then DMA partition-broadcast 1/sum back out.
                    tmp = rpool.tile([128, 512], f32, tag="tmp")
                    nc.vector.tensor_copy(tmp[64:65, :], ye[64:65, :])
                    nc.vector.tensor_copy(tmp[0:1, :], yo[0:1, :])
                    rs = rpool.tile([128, 8], f32, tag="rs")
                    nc.sync.dma_start(out=rs[:, 0:4], in_=tmp[64:65, :])
                    nc.sync.dma_start(out=rs[:, 4:8], in_=tmp[0:1, :])
                    rr = rpool.tile([128, 8], f32, tag="rr")
                    nc.vector.reciprocal(rr, rs)
                    rt = rpool.tile([128, 1024], f32, tag="rt")
                    nc.sync.dma_start(out=rt[0:1, 0:512], in_=rr[:, 0:4])
                    nc.sync.dma_start(out=rt[0:1, 512:1024], in_=rr[:, 4:8])
                    bsbE = rpool.tile([128, 512], f32, tag="bsbE")
                    bsbO = rpool.tile([128, 512], f32, tag="bsbO")
                    nc.gpsimd.partition_broadcast(bsbE[:, :], rt[0:1, 0:512])
                    nc.gpsimd.partition_broadcast(bsbO[:, :], rt[0:1, 512:1024])
                    nc.vector.tensor_mul(
                        yT_sb[0:64, g, qs:qs + 512], ye[0:64, :], bsbE[0:64, :]
                    )
                    nc.vector.tensor_mul(
                        yT_sb[64:128, g, qs:qs + 512], yo[64:128, :], bsbO[64:128, :]
                    )

            # phase D: out projection
            for t0 in range(NT):
                o_sb = opool.tile([128, C], f32, tag="o")
                pd0 = psS.tile([128, 512], f32, tag="s")
                pd1 = psS.tile([128, 512], f32, tag="s")
                for g in range(2):
                    nc.tensor.matmul(
                        pd0,
                        lhsT=(yT_sb[:, g, t0 * 128:(t0 + 1) * 128]),
                        rhs=(wo_sb[:, g, 0:512]),
                        start=(g == 0), stop=(g == 1),
                    )
                    nc.tensor.matmul(
                        pd1,
                        lhsT=(yT_sb[:, g, t0 * 128:(t0 + 1) * 128]),
                        rhs=(wo_sb[:, g, 512:1024]),
                        start=(g == 0), stop=(g == 1),
                    )
                nc.vector.tensor_add(o_sb[:, 0:512], pd0, bob_sb[:, 0:512])
                nc.vector.tensor_add(o_sb[:, 512:1024], pd1, bob_sb[:, 512:1024])
                nc.sync.dma_start(out=out[t0 * 128:(t0 + 1) * 128, :], in_=o_sb)

    nc.compile()
    return nc


def _host_consts():
    w = np.arange(896)[None, :]
    p = np.arange(128)[:, None]
    mskB = np.where(p >= w - 384, 0.0, NEG).astype(np.float32)
    vones = np.zeros((128, 2, 195), dtype=np.float16)
    vones[:, :, 64] = 1.0  # even-head ones column
    vones[:, :, 65] = 1.0  # odd-head ones column (block col 0)
    return mskB, vones


def make_in_maps(x, Wqkv, bqkv, Wo, bo):
    x = np.ascontiguousarray(np.asarray(x, dtype=np.float32))
    Wqkv = np.asarray(Wqkv, dtype=np.float32)
    bqkv = np.asarray(bqkv, dtype=np.float32)
    Wo = np.asarray(Wo, dtype=np.float32)
    bo = np.asarray(bo, dtype=np.float32)
    mskB, vones = _host_consts()
    in_maps = []
    for core in range(N_CORES):
        b, hg = divmod(core, 4)
        s = HC * hg
        bob = np.broadcast_to(bo, (128, C)) if hg == 0 else np.zeros((128, C), np.float32)
        in_maps.append({
            "xb": x[b].astype(np.float16),
            "wq": np.ascontiguousarray(Wqkv[:, s:s + HC]).astype(np.float16),
            "wk": np.ascontiguousarray(Wqkv[:, C + s:C + s + HC]).astype(np.float16),
            "wv": np.ascontiguousarray(Wqkv[:, 2 * C + s:2 * C + s + HC]).astype(np.float16),
            "bqs": np.ascontiguousarray(bqkv[s:s + HC]) * np.float32(SCALE),
            "bk": np.ascontiguousarray(bqkv[C + s:C + s + HC]),
            "bvb": np.ascontiguousarray(
                np.broadcast_to(bqkv[2 * C + s:2 * C + s + HC], (128, HC))
            ),
            "wo": np.ascontiguousarray(Wo[s:s + HC, :]).astype(np.float16),
            "bob": np.ascontiguousarray(bob),
            "mskB": mskB,
            "vones": vones,
        })
    return in_maps


def unshard(results):
    out = np.empty((B, T, C), dtype=np.float32)
    for b in range(B):
        acc = results[4 * b]["out"].astype(np.float32)
        for hg in range(1, 4):
            acc = acc + results[4 * b + hg]["out"]
        out[b] = acc
    return out


def get_nc():
    if "nc" not in _CACHE:
        _CACHE["nc"] = _build_nc()
    return _CACHE["nc"]


def kernel(x, Wqkv, bqkv, Wo, bo):
    from concourse.bass_utils import run_bass_kernel_spmd

    nc = get_nc()
    in_maps = make_in_maps(x, Wqkv, bqkv, Wo, bo)
    res = run_bass_kernel_spmd(nc, in_maps, list(range(N_CORES)))
    return unshard(res.results)



# revision 2
# speedup vs baseline: 1.6468x; 1.6468x over previous
"""Causal self-attention (flipped mask: attend to k >= q) on 8 Trainium2 cores.

Sharding: 2-way data parallel over batch x 4-way head parallel (4 heads/core).
Each core computes x[b] -> qkv (its 4 heads) -> attention -> partial out-proj
(its 256 rows of Wo); the host sums the 4 partials per batch (tensor-parallel
unshard) to produce the full [B, T, C] output.

v2 changes vs baseline:
  - x is transposed on the HOST; xT [C, T] f16 is DMA'd straight into SBUF
    (kills the 8x2.45us on-device DMA-transpose phase and its serialization).
  - x loaded in 4 t-slabs matching phase-B consumption order.
  - scores for the even/odd head of a pair land in ONE [128,1024] 2-bank
    PSUM tile; softmax exp is ONE scalar-engine instruction per j (halves
    ACT instruction overhead; ACT is the phase-C bottleneck).
  - no additive -1e5 mask on PSUM: scores are O(1) so exp never overflows
    f16. Diagonal-band tiles get a fixed [128,128] 0/1 triangular mask
    multiply (f16 2x DVE mode) + gpsimd memset of the fully-masked strip;
    exp width on band tiles is trimmed to the non-masked prefix.
  - output DMA'd as f16 per t-tile (halves out traffic; host sums in f32).
"""

import numpy as np

B, T, C = 2, 2048, 1024
H = 16
D = 64
NH = 4           # heads per core
HC = NH * D      # 256 local head cols
SCALE = 0.125    # 1/sqrt(D)
N_CORES = 8

NT = T // 128    # 16 t-tiles
NCC = C // 128   # 8 c-chunks
NQ = T // 512    # 4 q-chunks of 512
NJ = T // 128    # 16 kt-chunks of 128

_CACHE = {}


def _build_nc():
    import concourse.tile as tile
    from concourse import bacc, mybir

    f32 = mybir.dt.float32
    f16 = mybir.dt.float16
    Exp = mybir.ActivationFunctionType.Exp
    Ident = mybir.ActivationFunctionType.Identity

    nc = bacc.Bacc(None, target_bir_lowering=False, debug=False)

    xbT = nc.dram_tensor("xbT", [C, T], f16, kind="ExternalInput")
    wq = nc.dram_tensor("wq", [C, HC], f16, kind="ExternalInput")
    wk = nc.dram_tensor("wk", [C, HC], f16, kind="ExternalInput")
    wv = nc.dram_tensor("wv", [C, HC], f16, kind="ExternalInput")
    bqs = nc.dram_tensor("bqs", [HC], f32, kind="ExternalInput")
    bk = nc.dram_tensor("bk", [HC], f32, kind="ExternalInput")
    bvb = nc.dram_tensor("bvb", [128, HC], f32, kind="ExternalInput")
    wo = nc.dram_tensor("wo", [HC, C], f16, kind="ExternalInput")
    bob = nc.dram_tensor("bob", [128, C], f32, kind="ExternalInput")
    tri01 = nc.dram_tensor("tri01", [128, 128], f16, kind="ExternalInput")
    out = nc.dram_tensor("out", [T, C], f16, kind="ExternalOutput")

    with tile.TileContext(nc) as tc, (
        tc.tile_pool(name="consts", bufs=1)) as consts, (
        tc.tile_pool(name="wts", bufs=1)) as wts, (
        tc.tile_pool(name="persist", bufs=1)) as persist:

        # ---- weights (needed first; phase B starts on these) ----
        wq_sb = wts.tile([128, NCC, HC], f16)
        nc.sync.dma_start(out=wq_sb, in_=wq.rearrange("(a p) n -> p a n", p=128))
        wk_sb = wts.tile([128, NCC, HC], f16)
        nc.sync.dma_start(out=wk_sb, in_=wk.rearrange("(a p) n -> p a n", p=128))
        wv_sb = wts.tile([128, NCC, HC], f16)
        nc.sync.dma_start(out=wv_sb, in_=wv.rearrange("(a p) n -> p a n", p=128))
        wo_sb = wts.tile([128, 2, C], f16)
        nc.sync.dma_start(out=wo_sb, in_=wo.rearrange("(a p) n -> p a n", p=128))

        # ---- constants ----
        tri_sb = consts.tile([128, 128], f16)
        nc.sync.dma_start(out=tri_sb, in_=tri01[:, :])
        bq_sb = consts.tile([128, 2], f32)
        nc.sync.dma_start(out=bq_sb, in_=bqs.rearrange("(a p) -> p a", p=128))
        bk_sb = consts.tile([128, 2], f32)
        nc.sync.dma_start(out=bk_sb, in_=bk.rearrange("(a p) -> p a", p=128))
        bvb_sb = consts.tile([128, NH, D], f32)
        nc.sync.dma_start(out=bvb_sb, in_=bvb.rearrange("p (h d) -> p h d", h=NH))
        bob_sb = consts.tile([128, C], f32)
        nc.sync.dma_start(out=bob_sb, in_=bob[:, :])

        # ---- persistent activations ----
        xT_sb = persist.tile([128, NCC, T], f16)
        xTr = xbT.rearrange("(a p) t -> p a t", p=128)
        # 4 t-slabs in phase-B consumption order (m-chunk major)
        for m in range(NQ):
            nc.sync.dma_start(
                out=xT_sb[:, :, m * 512:(m + 1) * 512],
                in_=xTr[:, :, m * 512:(m + 1) * 512],
            )

        qT_sb = persist.tile([128, 2, T], f16)   # [2 head-pair chunks, T]
        kT_sb = persist.tile([128, 2, T], f16)
        # v, augmented: per t-tile, per pair g: [65 even | 130 odd]
        # even block: cols 0..63 = v(2g), col 64 = 1.0
        # odd block:  col 0 = 1.0 (tile col 65), cols 64..127 = v(2g+1)
        v_sb = persist.tile([128, NT, 2, 195], f16)
        yT_sb = persist.tile([128, 2, T], f16)

        # ones columns for the folded softmax denominator (cols 66..128 and
        # 193..194 feed junk output partitions that are never read)
        for t0 in range(NT):
            nc.gpsimd.memset(v_sb[:, t0, :, 64:66], 1.0)

        # ---- phase B: projections ----
        with tc.tile_pool(name="psB", bufs=4, space="PSUM") as psB:
            # Interleave one qk chain (N=512, MM hides its own LDW) with one
            # v chain (N=256, LDW of the per-(t0,c0) xT stationary would
            # otherwise be exposed) so weight loads hide under matmuls.
            for i in range(NT):
                gm, is_k = divmod(i, 2)
                g, m = divmod(gm, NQ)
                w_sb = wk_sb if is_k else wq_sb
                t0 = i
                psqk = psB.tile([128, 512], f32, tag="pj")
                psv = psB.tile([128, 512], f32, tag="pj")
                for c0 in range(NCC):
                    nc.tensor.matmul(
                        psqk,
                        lhsT=(w_sb[:, c0, g * 128:(g + 1) * 128]),
                        rhs=(xT_sb[:, c0, m * 512:(m + 1) * 512]),
                        start=(c0 == 0), stop=(c0 == NCC - 1),
                    )
                    nc.tensor.matmul(
                        psv[:, 0:HC],
                        lhsT=(xT_sb[:, c0, t0 * 128:(t0 + 1) * 128]),
                        rhs=(wv_sb[:, c0, :]),
                        start=(c0 == 0), stop=(c0 == NCC - 1),
                    )
                if is_k:
                    nc.scalar.activation(
                        kT_sb[:, g, m * 512:(m + 1) * 512], psqk, Ident,
                        bias=bk_sb[:, g:g + 1], scale=1.0,
                    )
                else:
                    nc.scalar.activation(
                        qT_sb[:, g, m * 512:(m + 1) * 512], psqk, Ident,
                        bias=bq_sb[:, g:g + 1], scale=SCALE,
                    )
                psv4 = psv[:, 0:HC].rearrange("p (h d) -> p h d", h=NH)
                for gg in range(2):
                    nc.vector.tensor_add(
                        v_sb[:, t0, gg, 0:64], psv4[:, 2 * gg, :], bvb_sb[:, 2 * gg, :]
                    )
                    nc.vector.tensor_add(
                        v_sb[:, t0, gg, 129:193], psv4[:, 2 * gg + 1, :],
                        bvb_sb[:, 2 * gg + 1, :],
                    )

        # ---- phases C/D ----
        with (
            tc.tile_pool(name="epool", bufs=6) as epool,
            tc.tile_pool(name="rpool", bufs=2) as rpool,
            tc.tile_pool(name="opool", bufs=2) as opool,
            tc.tile_pool(name="psS", bufs=2, space="PSUM") as psS,
            tc.tile_pool(name="psY", bufs=2, space="PSUM") as psY,
        ):
            # phase C: attention with transposed scores; even head in cols
            # 0:512 of a 2-bank tile, odd head in cols 512:1024
            for n in range(NQ):
                qs = n * 512
                for g in range(2):
                    yt = psY.tile([128, 1024], f32, tag="y")
                    ye = yt[:, 0:512]
                    yo = yt[:, 512:1024]

                    def emit_av(jj, eprev):
                        nc.tensor.matmul(
                            ye[0:65, :],
                            lhsT=(v_sb[:, jj, g, 0:65]),
                            rhs=(eprev[:, 0:512]),
                            start=(jj == 4 * n), stop=(jj == NJ - 1),
                        )
                        nc.tensor.matmul(
                            yo,
                            lhsT=(v_sb[:, jj, g, 65:193]),
                            rhs=(eprev[:, 512:1024]),
                            start=(jj == 4 * n), stop=(jj == NJ - 1),
                        )

                    lag = []
                    for j in range(4 * n, NJ):
                        b_i = j - 4 * n
                        band = b_i < 4
                        ks = j * 128
                        ps = psS.tile([128, 1024], f32, tag="s")
                        nc.tensor.matmul(
                            ps[:, 0:512],
                            lhsT=(kT_sb[0:64, g, ks:ks + 128]),
                            rhs=(qT_sb[0:64, g, qs:qs + 512]),
                            start=True, stop=True,
                        )
                        nc.tensor.matmul(
                            ps[:, 512:1024],
                            lhsT=(kT_sb[64:128, g, ks:ks + 128]),
                            rhs=(qT_sb[64:128, g, qs:qs + 512]),
                            start=True, stop=True,
                        )
                        if len(lag) >= 2:
                            emit_av(*lag.pop(0))
                        e = epool.tile([128, 1024], f16, tag="e")
                        if band:
                            w = 128 * (b_i + 1)
                            e3 = e.rearrange("p (h q) -> p h q", h=2)
                            ps3 = ps.rearrange("p (h q) -> p h q", h=2)
                            nc.scalar.activation(e3[:, :, 0:w], ps3[:, :, 0:w], Exp)
                            # triangular 0/1 mask on the diagonal 128-block
                            nc.vector.tensor_mul(
                                e[:, w - 128:w], e[:, w - 128:w], tri_sb
                            )
                            nc.vector.tensor_mul(
                                e[:, 512 + w - 128:512 + w], e[:, 512 + w - 128:512 + w],
                                tri_sb,
                            )
                            # zero the fully-masked strip
                            if w < 512:
                                nc.gpsimd.memset(e3[:, :, w:512], 0.0)
                        else:
                            nc.scalar.activation(e, ps, Exp)
                        lag.append((j, e))
                    for item in lag:
                        emit_av(*item)
                    # normalize: denominators sit on 1 PSUM partition each
                    # (even @64, odd @0).  Make the reciprocal cheap by
                    # DMA-reshaping [1,512] -> [128,4] (recip cost scales with
                    # free size), then DMA partition-broadcast 1/sum back out.
                    tmp = rpool.tile([128, 512], f32, tag="tmp")
                    nc.vector.tensor_copy(tmp[64:65, :], ye[64:65, :])
                    nc.vector.tensor_copy(tmp[0:1, :], yo[0:1, :])
                    rs = rpool.tile([128, 8], f32, tag="rs")
                    nc.sync.dma_start(out=rs[:, 0:4], in_=tmp[64:65, :])
                    nc.sync.dma_start(out=rs[:, 4:8], in_=tmp[0:1, :])
                    rr = rpool.tile([128, 8], f32, tag="rr")
                    nc.vector.reciprocal(rr, rs)
                    rt = rpool.tile([128, 1024], f32, tag="rt")
                    nc.sync.dma_start(out=rt[0:1, 0:512], in_=rr[:, 0:4])
                    nc.sync.dma_start(out=rt[0:1, 512:1024], in_=rr[:, 4:8])
                    bsbE = rpool.tile([128, 512], f32, tag="bsbE")
                    bsbO = rpool.tile([128, 512], f32, tag="bsbO")
                    nc.gpsimd.partition_broadcast(bsbE[:, :], rt[0:1, 0:512])
                    nc.gpsimd.partition_broadcast(bsbO[:, :], rt[0:1, 512:1024])
                    nc.vector.tensor_mul(
                        yT_sb[0:64, g, qs:qs + 512], ye[0:64, :], bsbE[0:64, :]
                    )
                    nc.vector.tensor_mul(
                        yT_sb[64:128, g, qs:qs + 512], yo[64:128, :], bsbO[64:128, :]
                    )

            # phase D: out projection, f16 output streamed per t-tile
            for t0 in range(NT):
                o_sb = opool.tile([128, C], f16, tag="o")
                pd = psS.tile([128, 1024], f32, tag="s")
                for g in range(2):
                    nc.tensor.matmul(
                        pd[:, 0:512],
                        lhsT=(yT_sb[:, g, t0 * 128:(t0 + 1) * 128]),
                        rhs=(wo_sb[:, g, 0:512]),
                        start=(g == 0), stop=(g == 1),
                    )
                    nc.tensor.matmul(
                        pd[:, 512:1024],
                        lhsT=(yT_sb[:, g, t0 * 128:(t0 + 1) * 128]),
                        rhs=(wo_sb[:, g, 512:1024]),
                        start=(g == 0), stop=(g == 1),
                    )
                nc.vector.tensor_add(o_sb[:, 0:512], pd[:, 0:512], bob_sb[:, 0:512])
                nc.vector.tensor_add(
                    o_sb[:, 512:1024], pd[:, 512:1024], bob_sb[:, 512:1024]
                )
                nc.sync.dma_start(out=out[t0 * 128:(t0 + 1) * 128, :], in_=o_sb)

    nc.compile()
    return nc


def _host_consts():
    p = np.arange(128)[:, None]
    c = np.arange(128)[None, :]
    tri01 = (p >= c).astype(np.float16)
    return tri01


def make_in_maps(x, Wqkv, bqkv, Wo, bo):
    x = np.asarray(x, dtype=np.float32)
    Wqkv = np.asarray(Wqkv, dtype=np.float32)
    bqkv = np.asarray(bqkv, dtype=np.float32)
    Wo = np.asarray(Wo, dtype=np.float32)
    bo = np.asarray(bo, dtype=np.float32)
    tri01 = _host_consts()
    xT = [np.ascontiguousarray(x[b].T).astype(np.float16) for b in range(B)]
    in_maps = []
    for core in range(N_CORES):
        b, hg = divmod(core, 4)
        s = HC * hg
        bob = np.broadcast_to(bo, (128, C)) if hg == 0 else np.zeros((128, C), np.float32)
        in_maps.append({
            "xbT": xT[b],
            "wq": np.ascontiguousarray(Wqkv[:, s:s + HC]).astype(np.float16),
            "wk": np.ascontiguousarray(Wqkv[:, C + s:C + s + HC]).astype(np.float16),
            "wv": np.ascontiguousarray(Wqkv[:, 2 * C + s:2 * C + s + HC]).astype(np.float16),
            "bqs": np.ascontiguousarray(bqkv[s:s + HC]) * np.float32(SCALE),
            "bk": np.ascontiguousarray(bqkv[C + s:C + s + HC]),
            "bvb": np.ascontiguousarray(
                np.broadcast_to(bqkv[2 * C + s:2 * C + s + HC], (128, HC))
            ),
            "wo": np.ascontiguousarray(Wo[s:s + HC, :]).astype(np.float16),
            "bob": np.ascontiguousarray(bob),
            "tri01": tri01,
        })
    return in_maps


def unshard(results):
    out = np.empty((B, T, C), dtype=np.float32)
    for b in range(B):
        acc = results[4 * b]["out"].astype(np.float32)
        for hg in range(1, 4):
            acc = acc + results[4 * b + hg]["out"].astype(np.float32)
        out[b] = acc
    return out


def get_nc():
    if "nc" not in _CACHE:
        _CACHE["nc"] = _build_nc()
    return _CACHE["nc"]


def kernel(x, Wqkv, bqkv, Wo, bo):
    from concourse.bass_utils import run_bass_kernel_spmd

    nc = get_nc()
    in_maps = make_in_maps(x, Wqkv, bqkv, Wo, bo)
    res = run_bass_kernel_spmd(nc, in_maps, list(range(N_CORES)))
    return unshard(res.results)
